# revision 31
# baseline (speedup 1.0000x reference)
"""Multi-head causal attention (B=2, S=2048, d_model=1024, H=16) on 8 Trainium2
NeuronCores.

Sharding: core c -> batch b = c // 4, head group g = c % 4 (heads 4g..4g+3).
Data-parallel over the batch, tensor-parallel over heads: each core computes
QKV projections for its 4 heads (column-sliced Wqkv), causal attention for
those heads, and a partial output projection (row-sliced Wo). The host sums
the 4 partial outputs per batch and adds the output bias.

Per-core dataflow (activations bf16, PSUM fp32), fully software-pipelined:

* ALL inputs ride in one flat bf16 blob (NRT tensor-bind cost is
  per-tensor per-rep); f32 biases are bit-packed as bf16 pairs and land in
  bitcast SBUF views.
* x streams in four 512-column chunks (one 3-dim DMA per half-k-range) into
  a single [128, 8, 2048] tile; w in two column halves per queue so the
  first QKV groups start ~1us after launch.
* Attention runs query chunks in ASCENDING order, so chunk c needs only
  QKV(0..c): the ScalarE exp stream (the dominant ~75us activation load)
  starts ~10us in, not after the whole projection phase.
* A filler scheduler interleaves QKV / output-projection micro-units (2-8
  matmuls) into the attention tile loop: one unit is pumped before each
  key tile, so the PE fills the slack the exp-paced attention leaves.
  Required units are front-pushed before their dependent pair (v before
  diagonal tiles, mt1/mt3 before the hp1 pair, next chunk's mt0/mt2 during
  hp1); output-projection units go to a deferrable queue that keeps the
  filler-starved final chunk fed.

Per key tile both heads' scores land in one fused [128, 2, 512] PSUM tile
(even head on PE row group 0-63, odd head 64-127 via auto tile_position);
a single 1024-element exp on ScalarE (~1.04us, the pacing op) frees both
banks at once. The causal mask is a PE accumulation (-1e9 * I @ trim into
the diagonal block pre-exp).

AV runs TRANSPOSED: stationary = pT query-slice [128 keys, 128 queries],
moving = v [128 keys, 64+1] -> av2 [128 queries, 65] uses the full PE array
width (half the row count of the natural layout), and the ones-column
denominator l lands as a per-PARTITION scalar. The drain is pure engine
work, no DMA bounces: DVE reciprocal [128,1] -> DVE tensor_scalar
normalize with 1/l as a per-partition scalar (kept OFF ScalarE: any small
op inserted into the in-order exp stream stretches the exp cadence and
stalls PE scores via sT-slot recycling; only the final pair, whose exps
are done, borrows ScalarE) -> PE transpose via identity (tile_position
col 64 puts odd heads straight onto partitions 64-127) -> copy into the
[dims, queries] values tile the output projection contracts. PSUM accumulation groups are
per-bank: av2's four query-slice regions share one group (start marks the
whole 2KB zero region; sub-region first-writes replace-after-zero).

PSUM budget (8 banks exactly): work pool 2 (QKV pq/pv, projection po, and
drain transposes), scores sT 2x[128,2,512] (4), av2 2.
"""

import sys

sys.path.insert(0, "/opt/trn_rl_repo")

import numpy as np

import concourse.bass as bass
import concourse.mybir as mybir
import concourse.tile as tile
from concourse.bass_utils import run_bass_kernel_spmd

F32 = mybir.dt.float32
F32R = mybir.dt.float32r
BF16 = mybir.dt.bfloat16

B, S, D = 2, 2048, 1024
H_TOT = 16
HD = 64
H_PER_CORE = 4
N_CORES = 8
SCALE = 1.0 / np.sqrt(HD)

ST = S // 128   # 16 key tiles of 128
NCH = S // 512  # 4 query chunks of 512

# All non-x inputs ride in one flat bf16 blob: NRT tensor-binding overhead
# is per-tensor per-rep (~17us/tensor through this bench path), so the
# kernel binds just two inputs (xT + blob). f32 payloads (the biases) are
# bit-packed as pairs of bf16 elements and DMA'd into bitcast SBUF views.
OFF_W = 0                              # [1024, 768] bf16
OFF_BQK = OFF_W + D * 768              # [128, 4] f32 -> [128, 8] bf16
OFF_BV = OFF_BQK + 128 * 8             # [256] f32 -> [512] bf16
OFF_EYENEG = OFF_BV + 512              # [128, 128] bf16
OFF_EYE = OFF_EYENEG + 128 * 128
OFF_TRIM = OFF_EYE + 128 * 128
OFF_WO = OFF_TRIM + 128 * 128          # [256, 1024] bf16
OFF_X = OFF_WO + 256 * D               # [1024, 2048] bf16 (x transposed)
BLOB_N = OFF_X + D * S


def _split_multi_waits(nc):
    """This container's walrus rejects >1 sem wait per instruction. Move
    extra waits onto fresh single-wait NOPs on the same engine, inserted
    immediately before the instruction (same-engine streams are in-order,
    so semantics are unchanged)."""
    n = 0
    for func in nc.m.functions:
        for bb in func.blocks:
            i = 0
            while i < len(bb.instructions):
                ins = bb.instructions[i]
                si = ins.sync_info
                if si is not None and si.on_wait and len(si.on_wait) > 1:
                    waits = list(si.on_wait)
                    si.on_wait = [waits[-1]]
                    eng = nc.engines[ins.engine]
                    nops = []
                    for w in waits[:-1]:
                        ni = eng.nop(nofuse=True, hint="wait_split").ins
                        if ni.sync_info is None:
                            ni.sync_info = mybir.SyncInfo(on_wait=[w], on_update=[])
                        else:
                            ni.sync_info.on_wait = [w]
                        nops.append(ni)
                    for ni in nops:
                        for f2 in nc.m.functions:
                            for bb2 in f2.blocks:
                                if ni in bb2.instructions:
                                    bb2.instructions.remove(ni)
                    for k, ni in enumerate(nops):
                        bb.instructions.insert(i + k, ni)
                    i += len(nops)
                    n += len(nops)
                i += 1
    return n


def build_bass():
    nc = bass.Bass()

    blob = nc.dram_tensor("blob", [BLOB_N], BF16, kind="ExternalInput")
    out = nc.dram_tensor("out", [S, D], BF16, kind="ExternalOutput")

    def blob_ap(off, ap):
        return bass.AP(tensor=blob, offset=off, ap=ap)

    # DMA trigger queues: SP-HWDGE (sync) + Pool-SWDGE (gpsimd) carry the
    # steady-state stream; ACT-HWDGE (scalar) is only safe while ScalarE is
    # idle (load phase) since triggers with sem waits would stall the exps.
    def dma_fast(out_ap, in_ap):
        # latency-critical small transfers: SP HWDGE (~625ns trigger)
        return nc.sync.dma_start(out_ap, in_ap)

    def dma_bulk(out_ap, in_ap):
        # throughput transfers: Pool SWDGE (Pool engine is otherwise idle)
        return nc.gpsimd.dma_start(out_ap, in_ap)

    with tile.TileContext(nc) as tc:
        with (
            tc.tile_pool(name="consts", bufs=1) as consts,
            tc.tile_pool(name="xw_p", bufs=1) as xw_p,
            tc.tile_pool(name="qkT_p", bufs=1) as qkT_p,
            tc.tile_pool(name="v_p", bufs=1) as v_p,
            tc.tile_pool(name="values_p", bufs=2) as values_p,
            tc.tile_pool(name="work_ps", bufs=2, space="PSUM") as work_ps,
            tc.tile_pool(name="sT_ps", bufs=2, space="PSUM") as sT_ps,
            tc.tile_pool(name="av_ps", bufs=2, space="PSUM") as av_ps,
            tc.tile_pool(name="pT_p", bufs=6) as pT_p,
            tc.tile_pool(name="lr_p", bufs=10) as lr_p,
            tc.tile_pool(name="v2t_p", bufs=8) as v2t_p,
            tc.tile_pool(name="out_p", bufs=6) as out_p,
        ):
            # ---- constants / persistent tiles ----
            bias_qk_sb = consts.tile([128, 4], F32)
            vbias_bc = consts.tile([128, 256], F32)
            wo_sb = [consts.tile([128, D], BF16, name=f"wo{i}") for i in range(2)]
            eyeneg_sb = consts.tile([128, 128], BF16)
            eye_sb = consts.tile([128, 128], BF16)
            trim_sb = consts.tile([128, 128], BF16)
            ones_hi = consts.tile([128, 64], F32R)
            nc.vector.memset(ones_hi[64:65, :].bitcast(F32), 1.0)
            # warmup: a 1-element Exp during the (ScalarE-idle) load phase
            # pulls the ~1.3us ACT_TABLE_LOAD off the attention-start
            # critical path (the first real exp otherwise stalls on it)
            warm_t = consts.tile([128, 4], F32)
            nc.scalar.activation(
                warm_t[64:65, 0:1],
                ones_hi[64:65, 0:1].bitcast(F32),
                mybir.ActivationFunctionType.Exp,
                scale=1.0,
            )

            # x_all[p, k, s] = xT[128k + p, s]; w_all[p, k, j] = w[128k + p, j]
            x_all = xw_p.tile([128, 8, S], BF16, name="x_all")
            w_all = xw_p.tile([128, 8, 768], BF16, name="w_all")
            qkT = [qkT_p.tile([128, S], BF16, name=f"qkT{mt}") for mt in range(4)]
            v_sb = [v_p.tile([128, H_PER_CORE, 65], BF16, name=f"v{st}") for st in range(ST)]
            values = [
                values_p.tile([128, S], BF16, name=f"vals{hp}", tag="vals")
                for hp in range(2)
            ]

            # ---- load program ----
            # w (k-pairs) interleaved with x chunk 0 so the k'th (w, x)
            # pair lands early and QKV(0) paces on arrivals; x chunks 1-3
            # follow. Queues: sync takes k 0-3, gpsimd k 4-7.
            def x_chunk_ap(klo, khi, ch):
                return blob_ap(
                    OFF_X + 128 * klo * S + 512 * ch,
                    [[S, 128], [128 * S, khi - klo], [1, 512]],
                )

            def w_ap_cols(klo, khi, jlo, jhi):
                return blob_ap(
                    OFF_W + 128 * klo * 768 + jlo,
                    [[768, 128], [128 * 768, khi - klo], [1, jhi - jlo]],
                )

            # the first QKV groups (mt0/mt2) only need w cols 0:384;
            # k-pair-granular pieces let the k-ordered accumulation start
            # after ~350KB instead of the full first-chunk load
            for eng, klo, khi in ((nc.sync, 0, 4), (nc.gpsimd, 4, 8)):
                for k2 in range(klo, khi, 2):
                    eng.dma_start(
                        w_all[:, k2 : k2 + 2, 0:384], w_ap_cols(k2, k2 + 2, 0, 384)
                    )
                    eng.dma_start(
                        x_all[:, k2 : k2 + 2, 0:512], x_chunk_ap(k2, k2 + 2, 0)
                    )
                eng.dma_start(w_all[:, klo:khi, 384:768], w_ap_cols(klo, khi, 384, 768))
            # constants on the ACT queue (idle until attention starts);
            # f32 biases land through bitcast views of the bf16 blob
            nc.scalar.dma_start(
                bias_qk_sb[:].bitcast(BF16), blob_ap(OFF_BQK, [[8, 128], [1, 8]])
            )
            nc.scalar.dma_start(
                vbias_bc[:].bitcast(BF16), blob_ap(OFF_BV, [[0, 128], [1, 512]])
            )
            nc.scalar.dma_start(eyeneg_sb[:], blob_ap(OFF_EYENEG, [[128, 128], [1, 128]]))
            nc.scalar.dma_start(eye_sb[:], blob_ap(OFF_EYE, [[128, 128], [1, 128]]))
            nc.scalar.dma_start(trim_sb[:], blob_ap(OFF_TRIM, [[128, 128], [1, 128]]))
            for i in range(2):
                nc.scalar.dma_start(
                    wo_sb[i][:], blob_ap(OFF_WO + 128 * i * D, [[D, 128], [1, D]])
                )
            for ch in range(1, NCH):
                nc.sync.dma_start(x_all[:, 0:4, 512 * ch : 512 * (ch + 1)], x_chunk_ap(0, 4, ch))
                nc.gpsimd.dma_start(x_all[:, 4:8, 512 * ch : 512 * (ch + 1)], x_chunk_ap(4, 8, ch))

            # ---- QKV for one 512-query chunk ----
            # mt 0/1 = q heads 01/23, mt 2/3 = k heads 01/23 (col-partition
            # layout via PE); v natural [seq, head, 64+1] with ones column.
            # Emission is unit-granular: the attention tile loop pumps one
            # unit before each key tile so QKV / output-projection matmuls
            # fill the PE gaps left by the exp-paced attention stream.
            from collections import deque

            filler = deque()
            late = deque()  # deferrable units (output projection) -- held
            # back so the filler-starved final chunk has PE work to absorb
            # its exp-paced slack

            def pump(n=1, late_ok=False):
                for _ in range(n):
                    if filler:
                        _tag, fn = filler.popleft()
                        fn()
                    elif late_ok and late:
                        _tag, fn = late.popleft()
                        fn()
                    else:
                        return

            def flush(tags):
                while any(t == tag for tag, _ in filler for t in tags):
                    _tag, fn = filler.popleft()
                    fn()

            def qk_units(mt, ch):
                state = {}

                def half(h):
                    def fn():
                        if h == 0:
                            state["pq"] = work_ps.tile([128, 512], F32, name="pq", tag="work")
                        pq = state["pq"]
                        for k in range(4 * h, 4 * h + 4):
                            nc.tensor.matmul(
                                pq[:],
                                w_all[:, k, 128 * mt : 128 * (mt + 1)],
                                x_all[:, k, 512 * ch : 512 * (ch + 1)],
                                start=(k == 0),
                                stop=(k == 7),
                            )
                        if h == 1:
                            nc.vector.tensor_scalar(
                                qkT[mt][:, 512 * ch : 512 * (ch + 1)],
                                pq[:],
                                bias_qk_sb[:, mt : mt + 1],
                                None,
                                mybir.AluOpType.add,
                            )
                    return fn

                return [(f"qk{mt}_{ch}", half(0)), (f"qk{mt}_{ch}", half(1))]

            def v_unit(st):
                def fn():
                    pv = work_ps.tile([128, 256], F32, name="pv", tag="work")
                    for k in range(8):
                        nc.tensor.matmul(
                            pv[:],
                            x_all[:, k, 128 * st : 128 * (st + 1)],
                            w_all[:, k, 512:768],
                            start=(k == 0),
                            stop=(k == 7),
                        )
                    nc.vector.memset(v_sb[st][:, :, 64:65], 1.0)
                    nc.vector.tensor_tensor(
                        v_sb[st][:, :, 0:64],
                        pv[:].rearrange("p (h d) -> p h d", h=H_PER_CORE),
                        vbias_bc[:].rearrange("p (h d) -> p h d", h=H_PER_CORE).bitcast(F32),
                        mybir.AluOpType.add,
                    )
                return (f"v{st}", fn)

            def po_unit(c0, st, nh, osb_box, last):
                def fn():
                    po = work_ps.tile([128, 512], F32, name="po", tag="work")
                    for hp2 in range(2):
                        nc.tensor.matmul(
                            po[:],
                            values[hp2][:, 128 * st : 128 * (st + 1)],
                            wo_sb[hp2][:, 512 * nh : 512 * (nh + 1)],
                            start=(hp2 == 0),
                            stop=(hp2 == 1),
                        )
                    if nh == 0:
                        osb_box.append(out_p.tile([128, 1024], BF16, name="o_sb"))
                    o_sb = osb_box[-1]
                    # the tail chunk's copies alternate DVE/ScalarE (exp
                    # stream is finished by then) and each half DMAs as
                    # soon as its copy lands; earlier chunks keep ScalarE
                    # exp-only and merge the two halves into one DMA
                    if last and nh == 1:
                        nc.scalar.copy(o_sb[:, 512 * nh : 512 * (nh + 1)], po[:])
                    else:
                        nc.vector.tensor_copy(o_sb[:, 512 * nh : 512 * (nh + 1)], po[:])
                    if last:
                        (dma_fast if nh else dma_bulk)(
                            out[
                                128 * st : 128 * (st + 1),
                                512 * nh : 512 * (nh + 1),
                            ],
                            o_sb[:, 512 * nh : 512 * (nh + 1)],
                        )
                    elif nh == 1:
                        dma_bulk(out[128 * st : 128 * (st + 1), :], o_sb[:])
                return (f"po{c0}", fn)

            def push_proj(c0, w=512, last=False):
                for st in range(c0 // 128, (c0 + w) // 128):
                    box = []
                    for nh in range(2):
                        late.append(po_unit(c0, st, nh, box, last))

            # ---- attention for one chunk (scores/exp/AV + per-slice drain) ----
            def emit_attn_pair(
                c0, hp, cw=512, act_drain=False, late_ok=False, act_slack=False
            ):
                q_t = qkT[hp]
                k_t = qkT[2 + hp]
                nqs = cw // 128
                av2 = [
                    av_ps.tile([128, nqs, 65], F32, name=f"av{c0}_{hp}_{hh}", tag="av")
                    for hh in range(2)
                ]
                n_jt = (c0 + cw) // 128

                def drain(hh, qs):
                    lsc = lr_p.tile([128, 1], F32, name="lsc", tag="lsc")
                    nc.vector.reciprocal(lsc[:, 0:1], av2[hh][:, qs, 64:65])
                    v2t = v2t_p.tile([128, 64], BF16, name="v2t")
                    # normalize: per-partition 1/l as a scalar. Odd head on
                    # ScalarE (activation Copy with scale AP) while the exp
                    # stream has slack, even head on DVE -- the two av2
                    # slots then recycle through independent engine streams.
                    if hh == 1 and (act_slack or act_drain):
                        nc.scalar.activation(
                            v2t[:],
                            av2[hh][:, qs, 0:64],
                            mybir.ActivationFunctionType.Copy,
                            scale=lsc[:, 0:1],
                        )
                    else:
                        nc.vector.tensor_scalar(
                            v2t[:],
                            av2[hh][:, qs, 0:64],
                            lsc[:, 0:1],
                            None,
                            mybir.AluOpType.mult,
                        )
                    vT = work_ps.tile([128, 128], BF16, name="vT", tag="work")
                    pr = 64 * hh
                    nc.tensor.transpose(
                        vT[pr : pr + 64, 0:128],
                        v2t[:],
                        eye_sb[:],
                        tile_position=(0, pr),
                    )
                    cp = nc.scalar.copy if act_drain else nc.vector.tensor_copy
                    cp(
                        values[hp][pr : pr + 64, c0 + 128 * qs : c0 + 128 * (qs + 1)],
                        vT[pr : pr + 64, 0:128],
                    )

                for jj in range(n_jt):
                    pump(1, late_ok=late_ok)
                    q0 = max(c0, 128 * jj)
                    cols = c0 + cw - q0
                    diag = 128 * jj >= c0
                    # one fused 2-bank tile for both heads of the pair:
                    # the single exp frees both heads' score banks at
                    # once; the A/B scores matmuls stay adjacent and
                    # pack onto disjoint PE row groups.
                    # always [.., 2, 512]: each head's scores own a full
                    # PSUM bank even for cw=256, keeping the accumulation
                    # groups per-bank (walrus rejects two groups per bank)
                    sT = sT_ps.tile([128, 2, 512], F32, name="sT", tag="sT")
                    pT = pT_p.tile([128, 2, 512], BF16, name="pT", tag="pT")
                    for hh in range(2):
                        hr = 64 * hh
                        nc.tensor.matmul(
                            sT[:, hh, 0:cols],
                            k_t[hr : hr + 64, 128 * jj : 128 * (jj + 1)],
                            q_t[hr : hr + 64, q0 : c0 + cw],
                            start=True,
                            stop=not diag,
                        )
                    if diag:
                        # causal mask: accumulate -1e9 into the
                        # sub-diagonal half of the 128-wide block
                        for hh in range(2):
                            nc.tensor.matmul(
                                sT[:, hh, 0:128],
                                eyeneg_sb[:],
                                trim_sb[:],
                                start=False,
                                stop=True,
                            )
                    nc.scalar.activation(
                        pT[:, :, 0:cols],
                        sT[:, :, 0:cols],
                        mybir.ActivationFunctionType.Exp,
                        scale=float(SCALE),
                    )
                    # transposed AV: av2[q, d] += pT[k, q].T @ v[k, d]; the
                    # ones column of v puts the denominator l at av2[q, 64].
                    # Query slice qs is only touched by key tiles jj <=
                    # c0/128 + qs (causality), so each slice stops -- and
                    # drains -- as soon as that tile retires.
                    for hh in range(2):
                        h = 2 * hp + hh
                        for qs in range(nqs):
                            if 128 * jj > c0 + 128 * qs:
                                continue
                            a = c0 + 128 * qs - q0
                            # one PSUM accumulation group per av2 bank: the
                            # start marks the whole 2KB zero region, so each
                            # qs sub-region's first write (all at jj=0)
                            # replaces-after-zero; stop on the bank's last
                            # write (last tile, last slice).
                            nc.tensor.matmul(
                                av2[hh][:, qs, 0:65],
                                pT[:, hh, a : a + 128],
                                v_sb[jj][:, h, 0:65],
                                start=(jj == 0 and qs == 0),
                                stop=(jj == n_jt - 1 and qs == nqs - 1),
                            )

                for qs in range(nqs):
                    drain(0, qs)
                    drain(1, qs)

            # ---- schedule ----
            # chunk 0: its tiles are all diagonal, so v(0) must fully
            # precede the attention; emit mt0/mt2/v direct.
            for tag, fn in qk_units(0, 0) + qk_units(2, 0):
                fn()
            for st in range(4):
                v_unit(st)[1]()
            filler.extend(qk_units(1, 0) + qk_units(3, 0))
            emit_attn_pair(0, 0)
            flush(["qk1_0", "qk3_0"])
            filler.extendleft(reversed(qk_units(0, 1) + qk_units(2, 1)))
            emit_attn_pair(0, 1)
            push_proj(0)

            for c in range(1, NCH):
                c0 = 512 * c
                last = c == NCH - 1
                flush([f"qk0_{c}", f"qk2_{c}"])
                # front-push this chunk's remaining prerequisites: v before
                # the diagonal tiles, mt1/mt3 before the hp1 pair.
                pre = [v_unit(st) for st in range(4 * c, 4 * c + 4)]
                pre += qk_units(1, c) + qk_units(3, c)
                filler.extendleft(reversed(pre))
                emit_attn_pair(c0, 0, late_ok=True)
                flush([f"qk1_{c}", f"qk3_{c}"])
                if not last:
                    filler.extendleft(
                        reversed(qk_units(0, c + 1) + qk_units(2, c + 1))
                    )
                emit_attn_pair(c0, 1, act_drain=last, late_ok=True)
                if not last:
                    push_proj(c0)

            for q in (filler, late):
                while q:
                    _tag, fn = q.popleft()
                    fn()
            push_proj(512 * (NCH - 1), last=True)
            while late:
                _tag, fn = late.popleft()
                fn()

    _split_multi_waits(nc)
    return nc


_NC_CACHE = None


def _get_nc():
    global _NC_CACHE
    if _NC_CACHE is None:
        _NC_CACHE = build_bass()
    return _NC_CACHE


def make_in_maps(x, mask, Wqkv, bqkv, Wo, bo):
    x = np.asarray(x, dtype=np.float32)
    Wqkv = np.asarray(Wqkv, dtype=np.float32)
    bqkv = np.asarray(bqkv, dtype=np.float32)
    Wo = np.asarray(Wo, dtype=np.float32)

    import ml_dtypes

    xT = [np.ascontiguousarray(x[b].T).astype(ml_dtypes.bfloat16) for b in range(B)]
    eyeneg = (np.eye(128, dtype=np.float32) * -1e9).astype(ml_dtypes.bfloat16)
    eye = np.eye(128, dtype=np.float32).astype(ml_dtypes.bfloat16)
    # trim[k, q] = 1 iff query q precedes key k (masked)
    trim = (np.arange(128)[None, :] < np.arange(128)[:, None]).astype(ml_dtypes.bfloat16)

    in_maps = []
    for c in range(N_CORES):
        b, g = c // 4, c % 4
        heads = [4 * g + h for h in range(H_PER_CORE)]
        # Wqkv columns are per-head interleaved: head H -> q cols
        # 192H..192H+64, k cols 192H+64.., v cols 192H+128..
        iq = np.concatenate([np.arange(192 * H, 192 * H + 64) for H in heads])
        ik = np.concatenate([np.arange(192 * H + 64, 192 * H + 128) for H in heads])
        iv = np.concatenate([np.arange(192 * H + 128, 192 * H + 192) for H in heads])
        w_c = np.ascontiguousarray(
            np.concatenate([Wqkv[:, iq], Wqkv[:, ik], Wqkv[:, iv]], axis=1)
        ).astype(ml_dtypes.bfloat16)
        bias_qk = np.ascontiguousarray(
            np.stack(
                [bqkv[iq[:128]], bqkv[iq[128:]], bqkv[ik[:128]], bqkv[ik[128:]]],
                axis=1,
            ).astype(np.float32)
        )
        bias_v = np.ascontiguousarray(bqkv[iv].astype(np.float32))
        wo_c = np.ascontiguousarray(Wo[256 * g : 256 * (g + 1), :]).astype(
            ml_dtypes.bfloat16
        )
        blob = np.empty(BLOB_N, dtype=ml_dtypes.bfloat16)
        blob[OFF_X : OFF_X + D * S] = xT[b].ravel()
        blob[OFF_W : OFF_W + D * 768] = w_c.ravel()
        blob[OFF_BQK : OFF_BQK + 128 * 8] = bias_qk.view(ml_dtypes.bfloat16).ravel()
        blob[OFF_BV : OFF_BV + 512] = bias_v.view(ml_dtypes.bfloat16).ravel()
        blob[OFF_EYENEG : OFF_EYENEG + 128 * 128] = eyeneg.ravel()
        blob[OFF_EYE : OFF_EYE + 128 * 128] = eye.ravel()
        blob[OFF_TRIM : OFF_TRIM + 128 * 128] = trim.ravel()
        blob[OFF_WO : OFF_WO + 256 * D] = wo_c.ravel()
        in_maps.append({"blob": blob})
    return in_maps


def bench(x, mask, Wqkv, bqkv, Wo, bo, iters=20):
    """Steady-state timing of the NEFF execution via PJRT with
    device-resident inputs. Returns (best_ns, all_ns)."""
    import time

    import jax
    import jax.numpy as jnp
    from jax.sharding import Mesh, PartitionSpec
    from jax.experimental.shard_map import shard_map
    from concourse import bass2jax
    from concourse.bass2jax import _bass_exec_p, install_neuronx_cc_hook

    install_neuronx_cc_hook()
    nc = _get_nc()
    in_maps = make_in_maps(x, mask, Wqkv, bqkv, Wo, bo)

    partition_name = nc.partition_id_tensor.name if nc.partition_id_tensor else None
    in_names, out_names, out_avals, zero_shapes = [], [], [], []
    for alloc in nc.m.functions[0].allocations:
        if not isinstance(alloc, mybir.MemoryLocationSet):
            continue
        name = alloc.memorylocations[0].name
        if alloc.kind == "ExternalInput":
            if name != partition_name:
                in_names.append(name)
        elif alloc.kind == "ExternalOutput":
            out_names.append(name)
            shape = tuple(alloc.tensor_shape)
            dtype = mybir.dt.np(alloc.dtype)
            out_avals.append(jax.core.ShapedArray(shape, dtype))
            zero_shapes.append((shape, dtype))
    n_params = len(in_names)
    n_outs = len(out_avals)
    all_in_names = list(in_names) + list(out_names)
    if partition_name is not None:
        all_in_names.append(partition_name)

    def _body(*args):
        operands = list(args)
        if partition_name is not None:
            operands.append(bass2jax.partition_id_tensor())
        outs = _bass_exec_p.bind(
            *operands,
            out_avals=tuple(out_avals),
            in_names=tuple(all_in_names),
            out_names=tuple(out_names),
            lowering_input_output_aliases=(),
            sim_require_finite=True,
            sim_require_nnan=True,
            nc=nc,
        )
        return tuple(outs)

    devices = jax.devices()[:N_CORES]
    mesh = Mesh(np.asarray(devices), ("core",))
    donate = tuple(range(n_params, n_params + n_outs))
    sharded = jax.jit(
        shard_map(
            _body,
            mesh=mesh,
            in_specs=(PartitionSpec("core"),) * (n_params + n_outs),
            out_specs=(PartitionSpec("core"),) * n_outs,
            check_rep=False,
        ),
        donate_argnums=donate,
        keep_unused=True,
    )

    concat_in = [
        np.concatenate([np.asarray(in_maps[c][in_names[i]]) for c in range(N_CORES)], axis=0)
        for i in range(n_params)
    ]
    sharding = jax.sharding.NamedSharding(mesh, PartitionSpec("core"))
    dev_in = [jax.device_put(a, sharding) for a in concat_in]

    def make_zeros():
        return [
            jax.device_put(
                np.zeros((N_CORES * s[0], *s[1:]), dt), sharding
            )
            for (s, dt) in zero_shapes
        ]

    # Async python-level chaining: each call donates the previous call's
    # outputs as its output buffers; calls pipeline on the device and we
    # only block at the end. Marginal time over the rep count isolates
    # per-execution device time from fixed RPC/dispatch overhead.
    def timed(reps):
        ts = []
        for _ in range(iters):
            outs = make_zeros()
            for z in outs:
                z.block_until_ready()
            t0 = time.perf_counter()
            for _ in range(reps):
                outs = sharded(*dev_in, *outs)
            for o in outs:
                o.block_until_ready()
            ts.append((time.perf_counter() - t0) * 1e9)
        return ts

    r_lo, r_hi = 1, 129
    t_lo = timed(r_lo)
    t_hi = timed(r_hi)
    best = (min(t_hi) - min(t_lo)) / (r_hi - r_lo)
    med = (sorted(t_hi)[len(t_hi) // 2] - sorted(t_lo)[len(t_lo) // 2]) / (r_hi - r_lo)
    return best, {"lo": t_lo, "hi": t_hi, "marginal_best": best, "marginal_med": med}


def kernel(x, mask, Wqkv, bqkv, Wo, bo, _trace=False):
    nc = _get_nc()
    in_maps = make_in_maps(x, mask, Wqkv, bqkv, Wo, bo)
    res = run_bass_kernel_spmd(nc, in_maps, core_ids=list(range(N_CORES)), trace=_trace)
    partials = [np.asarray(r["out"], dtype=np.float32) for r in res.results]
    bo = np.asarray(bo, dtype=np.float32)
    out = np.empty((B, S, D), dtype=np.float32)
    for b in range(B):
        out[b] = partials[4 * b] + partials[4 * b + 1] + partials[4 * b + 2] + partials[4 * b + 3] + bo
    if _trace:
        return out, res
    return out
